# revision 1
# baseline (speedup 1.0000x reference)
"""Trainium2 Bass kernel for nn_CAGetBoard (neural CA step).

Takes FULL inputs, shards batch across 8 NeuronCores (pure data parallel),
runs a Bass/Tile kernel per core, gathers the FULL output.

Per-core pipeline (B/8 images each), all matmuls bf16:
  - conv1 (Sobel folded into a 16->128 3x3 conv) = 2 accumulating matmuls
    (K=48 left-tap + K=96 center/right) over a 6-copy row/col-shifted
    stacked x (bf16, cast once to a DRAM scratch); 258-stride rows with
    zeroed pad columns give W-edge SAME padding via rhs free offsets.
  - relu+bias drains PSUM->SBUF bf16 (ACT/DVE 3:1), paired 2-chunk drains.
  - mm2 (128->16) col-tiled x4 with duplicated weights (M=32 at col bases
    0/32/64/96) -> packed [128,512] PSUM -> single tanh(+bias) drain; the
    d channel-3 rows stream back to row-layout tiles for the alive mask.
  - alive masks in row-layout; 3x3 binary dilation via banded bf16 matmuls
    + horizontal adds; u/alive replicated to the packed channel layout by
    SWDGE broadcast DMAs.
  - finishing: boards = clip_or_id((x + d*u) * alive) via full-width TTs
    + one fused min/max tensor_scalar.
"""

import numpy as np

import concourse.bass as bass
import concourse.bacc as bacc
import concourse.tile as tile
import concourse.mybir as mybir
from concourse.bass_utils import run_bass_kernel_spmd

dt = mybir.dt
F32 = dt.float32
F32R = dt.float32r
BF16 = dt.bfloat16
AF = mybir.ActivationFunctionType
OP = mybir.AluOpType

N_CORES = 8
C = 16
H = 256
W = 256
TR = 32                    # rows per compute block
WS = W + 2                 # padded row stride
N_BLK = H // TR
N_CHUNK = TR // 2          # 512-px chunks per block
N_GRP = N_CHUNK // 4       # mm2 groups per block
PX_IMG = H * W
EPS = 0.5
ALIVE_T = 0.1


def _build_consts(w1, b1, w2, b2):
    w1 = np.asarray(w1, np.float32)
    w2 = np.asarray(w2, np.float32)
    sob = np.array([[-1., 0., 1.], [-2., 0., 2.], [-1., 0., 1.]], np.float32)
    W1x, W1gx, W1gy = w1[:, 0:16], w1[:, 16:32], w1[:, 32:48]
    k1f = (W1gx[:, :, None, None] * sob[None, None, :, :]
           + W1gy[:, :, None, None] * sob.T[None, None, :, :])
    k1f[:, :, 1, 1] += W1x
    lhs = np.transpose(k1f, (3, 2, 1, 0)).reshape(3, 48, 128)
    lhsA = lhs[0].copy()
    lhsB = np.concatenate([lhs[1], lhs[2]], axis=0)

    w2dup = np.zeros((128, 32), np.float32)
    w2dup[:, 0:16] = w2.T
    w2dup[:, 16:32] = w2.T

    b2dup = np.zeros((128, 1), np.float32)
    for i in range(4):
        for d in range(2):
            s = 32 * i + 16 * d
            b2dup[s:s + 16, 0] = b2

    ones4 = np.zeros((4, 128), np.float32)
    for k in range(4):
        ones4[k, 32 * k:32 * k + 32] = 1.0

    bandB = np.zeros((128, 128), np.float32)
    for k in range(128):
        bandB[k, max(0, k - 1):k + 2] = 1.0
    bandClo = np.zeros((128, 128), np.float32)
    bandClo[0, 127] = 1.0
    bandChi = np.zeros((128, 128), np.float32)
    bandChi[127, 0] = 1.0
    clo1 = np.zeros((1, 128), np.float32)
    clo1[0, 127] = 1.0

    return dict(
        lhsA=lhsA, lhsB=lhsB,
        w2dup=w2dup, b1c=np.asarray(b1, np.float32).reshape(128, 1),
        b2dup=b2dup, ones4=ones4,
        bandB=bandB, bandClo=bandClo, bandChi=bandChi, clo1=clo1,
    )


CONST_SPECS = dict(
    lhsA=([48, 128], BF16), lhsB=([96, 128], BF16),
    w2dup=([128, 32], BF16), b1c=([128, 1], F32), b2dup=([128, 1], F32),
    ones4=([4, 128], BF16),
    bandB=([128, 128], BF16), bandClo=([128, 128], BF16),
    bandChi=([128, 128], BF16), clo1=([1, 128], BF16),
)


def build_program(n_img, reps=1):
    nc = bacc.Bacc("TRN2", target_bir_lowering=False)

    x_d = nc.dram_tensor("x", [n_img, C, H, W], F32, kind="ExternalInput")
    rand_d = nc.dram_tensor("rand", [n_img, H, W], F32, kind="ExternalInput")
    cst_d = {k: nc.dram_tensor(k, sh, d, kind="ExternalInput")
             for k, (sh, d) in CONST_SPECS.items()}
    out_d = nc.dram_tensor("out", [n_img, C, H, W], F32, kind="ExternalOutput")
    alive_d = nc.dram_tensor("alivescr", [n_img, PX_IMG], BF16, kind="Internal")
    xbf_d = nc.dram_tensor("xbfscr", [n_img, C, H, W], BF16, kind="Internal")

    xf = x_d.ap().rearrange("b c h w -> b c (h w)")
    outf = out_d.ap().rearrange("b c h w -> b c (h w)")
    randf = rand_d.ap().rearrange("b h w -> b (h w)")

    with tile.TileContext(nc) as tc:
        xbf = xbf_d.ap().rearrange("b c h w -> b c (h w)")
        _emit(nc, tc, n_img, xf, randf, cst_d, outf, alive_d.ap(),
              xbf, reps)
    nc.compile()
    return nc


def _emit(nc, tc, n_img, xf, randf, cst_d, outf, alivef, xbf, reps=1):
    from contextlib import ExitStack
    ctx = ExitStack()

    def pool(name, bufs, **kw):
        return ctx.enter_context(tc.tile_pool(name=name, bufs=bufs, **kw))

    consts = pool("consts", 1)
    stackp = pool("stack", 1)
    hgrp_p = pool("hgrp", 4)
    dgrp_p = pool("dgrp", 6)
    fin_p = pool("fin", 2)
    fs_p = pool("fs", 4)
    row_p = pool("rows", 4)
    rowsm_p = pool("rowsm", 2)
    misc_p = pool("misc", 1)
    conv_ps = pool("convps", 2, space="PSUM")
    mask_ps = pool("maskps", 1, space="PSUM")
    mm2_ps = pool("mm2ps", 2, space="PSUM")

    cst = {}
    for k, (sh, d) in CONST_SPECS.items():
        t = consts.tile(sh, d, tag=k, name=k)
        nc.sync.dma_start(t[:], cst_d[k].ap())
        cst[k] = t

    zeros = misc_p.tile([128, 1024], F32, tag="zeros", name="zeros")
    nc.vector.memset(zeros[:], 0.0)

    stacks = []
    for s in range(3):
        st = stackp.tile([96, TR * WS], BF16, tag=f"stack{s}", name=f"stack{s}")
        st3 = st.rearrange("p (r j) -> p r j", j=WS)
        nc.vector.memset(st3[:, :, 0:1], 0.0)
        nc.vector.memset(st3[:, :, W + 1:W + 2], 0.0)
        stacks.append(st3)

    sdil = []
    for s in range(4):
        t = misc_p.tile([128, WS], F32, tag=f"sdil{s}", name=f"sdil{s}")
        nc.vector.memset(t[:, 0:1], 0.0)
        nc.vector.memset(t[:, W + 1:W + 2], 0.0)
        sdil.append(t)

    def dilate_half(half, b_main, extra_lhs, extra_rhs, out_t, sgrp=0):
        """out = dilate3x3(binary) for one 128-row half.
        vertical: bandB.T @ b_main + extra_lhs.T @ extra_rhs, then horizontal
        adds on a 258-padded drain tile, then > 0.5."""
        vs = mask_ps.tile([128, W], F32, tag="mask", name="vs")
        nc.tensor.matmul(vs[:], cst["bandB"][:], b_main[:],
                         start=True, stop=(extra_lhs is None))
        if extra_lhs is not None:
            nc.tensor.matmul(vs[:], extra_lhs, extra_rhs,
                             start=False, stop=True)
        s = sdil[2 * sgrp + half]
        nc.scalar.activation(s[:, 1:W + 1], vs[:], AF.Copy)
        t = rowsm_p.tile([128, W], F32, tag="dil_t", name="dil_t")
        nc.vector.tensor_add(t[:], s[:, 0:W], s[:, 2:W + 2])
        nc.vector.tensor_add(t[:], t[:], s[:, 1:W + 1])
        nc.vector.tensor_single_scalar(out_t[:], t[:], 0.5, OP.is_gt)

    for b in range(n_img):
        nc.gpsimd.dma_start(xbf[b], xf[b])

    for b in [i for _ in range(reps) for i in range(n_img)]:
        # ---------------- row-layout pre-pass ----------------
        x3row, randrow, bpre, prealive = [], [], [], []
        for half in range(2):
            xt = row_p.tile([128, W], F32, tag="x3row", name="x3row")
            nc.sync.dma_start(
                xt[:], xf[b, 3, half * 128 * W:(half + 1) * 128 * W]
                .rearrange("(p w) -> p w", w=W))
            x3row.append(xt)
            rt = row_p.tile([128, W], F32, tag="randrow", name="randrow")
            nc.sync.dma_start(
                rt[:], randf[b, half * 128 * W:(half + 1) * 128 * W]
                .rearrange("(p w) -> p w", w=W))
            randrow.append(rt)
            bt = row_p.tile([128, W], BF16, tag="bpre", name="bpre")
            nc.vector.tensor_single_scalar(bt[:], xt[:], ALIVE_T, OP.is_gt)
            bpre.append(bt)
            prealive.append(row_p.tile([128, W], BF16, tag="prealive", name="prealive"))
        dilate_half(0, bpre[0], cst["bandClo"][:], bpre[1][:], prealive[0])
        dilate_half(1, bpre[1], cst["bandChi"][:], bpre[0][:], prealive[1])

        dkeep = {}
        d3row = []
        for half in range(2):
            t = row_p.tile([128, W], BF16, tag="d3row", name="d3row")
            d3row.append(t)

        def compute_block(blk):
            r0 = blk * TR
            dgb = dgrp_p.tile([128, 2048], BF16, tag="d", name="d")
            dkeep[blk] = dgb
            st3 = stacks[blk % 3]
            if blk == 0:
                nc.vector.memset(st3[0:32, 0:1, :], 0.0)
                nc.vector.memset(st3[32:64, 0:1, :], 0.0)
            if blk == N_BLK - 1:
                nc.vector.memset(st3[32:64, TR - 1:TR, :], 0.0)
                nc.vector.memset(st3[64:96, TR - 1:TR, :], 0.0)
            nc.vector.memset(st3[32:64, :, W:W + 1], 0.0)
            nc.vector.memset(st3[64:96, :, W:W + 1], 0.0)
            for di in range(3):
                rr_lo = max(0, 1 - di - r0)
                rr_hi = min(TR, H - r0 - di + 1)
                srcA = xbf[b, :, (r0 + rr_lo + di - 1) * W:
                           (r0 + rr_hi + di - 1) * W].rearrange(
                               "c (r w) -> c r w", w=W)
                englist = (nc.sync, nc.scalar)
                englist[di % 2].dma_start(
                    st3[16 * di:16 * di + 16, rr_lo:rr_hi, 1:W + 1], srcA)
                srcB = xbf[b, :, (r0 + rr_lo + di - 1) * W:
                           (r0 + rr_hi + di - 1) * W].rearrange(
                               "c (r w) -> c r w", w=W)[:, :, 1:W]
                englist[(di + 1) % 2].dma_start(
                    st3[48 + 16 * di:64 + 16 * di, rr_lo:rr_hi, 1:W], srcB)


            for g in range(N_GRP):
                hg = hgrp_p.tile([128, 2048], BF16, tag="hgrp", name="hgrp")
                for ip in range(2):
                    acc = conv_ps.tile([128, 1024], F32, tag="conv",
                                       name="conv")
                    for i in (2 * ip, 2 * ip + 1):
                        chk = 4 * i + g
                        asl = acc[:, 512 * (i - 2 * ip):512 * (i - 2 * ip + 1)]
                        nc.tensor.matmul(
                            asl, cst["lhsA"][:],
                            st3[0:48, 2 * chk:2 * chk + 2, 0:W],
                            start=True, stop=False)
                        nc.tensor.matmul(
                            asl, cst["lhsB"][:],
                            st3[0:96, 2 * chk:2 * chk + 2, 1:W + 1],
                            start=False, stop=True)
                    hsl = hg[:, 1024 * ip:1024 * (ip + 1)]
                    if (2 * g + ip) % 4 != 3:
                        nc.scalar.activation(hsl, acc[:], AF.Relu,
                                             bias=cst["b1c"][:, 0:1])
                    else:
                        nc.vector.scalar_tensor_tensor(
                            hsl, acc[:], cst["b1c"][:, 0:1], zeros[:],
                            op0=OP.add, op1=OP.max)
                mm = mm2_ps.tile([128, 512], F32, tag="mm2", name="mm2")
                for i in range(4):
                    nc.tensor.matmul(
                        mm[32 * i:32 * i + 32, :],
                        cst["w2dup"][:],
                        hg[:, 512 * i:512 * (i + 1)],
                        start=True, stop=True,
                        tile_position=(0, 32 * i))
                nc.scalar.activation(dgb[:, 512 * g:512 * (g + 1)], mm[:],
                                     AF.Tanh, bias=cst["b2dup"][:, 0:1])
            j0 = blk * N_CHUNK
            half = blk // 4
            for i in range(4):
                r = (2 * (j0 + 4 * i)) % 128
                nc.scalar.dma_start(d3row[half][r:r + 8, :],
                                    dgb[32 * i + 3:32 * i + 4, :])

        def post_binary(rows_ap_rand, rows_ap_x3, d3_ap, out_t):
            """out = (x3 + d3*(rand<eps)) > 0.1  on row-layout tiles."""
            m = rowsm_p.tile(list(out_t.shape), F32, tag="postm", name="postm")
            nc.vector.scalar_tensor_tensor(
                m[:], rows_ap_rand, EPS, d3_ap,
                op0=OP.is_lt, op1=OP.mult)
            nc.vector.tensor_add(m[:], m[:], rows_ap_x3)
            nc.vector.tensor_single_scalar(out_t[:], m[:], ALIVE_T, OP.is_gt)

        def alive_store(half, ar):
            nc.sync.dma_start(
                alivef[b, half * 128 * W:(half + 1) * 128 * W]
                .rearrange("(p w) -> p w", w=W), ar[:])

        def finish_block(blk):
            j0 = blk * N_CHUNK
            px0 = j0 * 512
            # x in dup layout: 4 DMAs of [32, 2048] (dup-pair folded)
            xd = fin_p.tile([128, 2048], F32, tag="xdup", name="xdup")
            for i in range(4):
                nc.sync.dma_start(
                    xd[32 * i:32 * i + 32, :],
                    xf[b, :, px0 + 2048 * i:px0 + 2048 * (i + 1)]
                    .unsqueeze(0).broadcast_to([2, C, 2048]))
            # u4 = (rand < 0.5) exact f32 cmp -> bf16 [4, 2048]
            r4 = fin_p.tile([4, 2048], F32, tag="r4", name="r4", bufs=1)
            nc.sync.dma_start(
                r4[:], randf[b, px0:px0 + 8192]
                .rearrange("(i n) -> i n", n=2048))
            u4 = fin_p.tile([4, 2048], BF16, tag="u4", name="u4")
            nc.vector.tensor_single_scalar(u4[:], r4[:], EPS, OP.is_lt)
            dgb = dkeep.pop(blk)
            t = fs_p.tile([128, 2048], F32, tag="fs", name="t")
            t2 = fs_p.tile([128, 2048], F32, tag="fs", name="t2")
            t3 = fs_p.tile([128, 2048], F32, tag="fs", name="t3")
            u16 = fin_p.tile([128, 2048], BF16, tag="u16", name="u16")
            a16 = fin_p.tile([128, 2048], BF16, tag="a16", name="a16")
            for i in range(4):
                nc.gpsimd.dma_start(
                    u16[32 * i:32 * i + 32, :],
                    u4[i:i + 1, :].unsqueeze(1)
                    .broadcast_to([1, 32, 2048]))
                nc.gpsimd.dma_start(
                    a16[32 * i:32 * i + 32, :],
                    alivef[b, px0 + 2048 * i:px0 + 2048 * (i + 1)]
                    .unsqueeze(0).broadcast_to([32, 2048]))
            nc.vector.tensor_mul(t[:], dgb[:], u16[:])
            nc.vector.tensor_add(t2[:], t[:], xd[:])
            nc.vector.tensor_mul(t3[:], t2[:], a16[:])
            t4 = fs_p.tile([128, 2048], F32, tag="fs", name="t4")
            nc.vector.tensor_scalar(t4[:], t3[:], 1.0, 0.0,
                                    op0=OP.min, op1=OP.max)
            for i in range(4):
                eng = nc.sync if i % 2 == 0 else nc.scalar
                eng.dma_start(
                    outf[b, 0:3, px0 + 2048 * i:px0 + 2048 * (i + 1)],
                    t4[32 * i:32 * i + 3, :])
                eng.dma_start(
                    outf[b, 3:16, px0 + 2048 * i:px0 + 2048 * (i + 1)],
                    t3[32 * i + 3:32 * i + 16, :])

        # ---------------- pipeline ----------------
        for blk in range(5):
            compute_block(blk)

        # post binary for half 0 + row 128 (first row of block 4)
        bpost0 = rowsm_p.tile([128, W], BF16, tag="bpost0", name="bpost0")
        post_binary(randrow[0][:], x3row[0][:], d3row[0][:], bpost0)
        bp128 = rowsm_p.tile([1, W], BF16, tag="bp128", name="bp128")
        post_binary(randrow[1][0:1, :], x3row[1][0:1, :], d3row[1][0:1, :],
                    bp128)

        postal0 = rowsm_p.tile([128, W], BF16, tag="postal0", name="postal0")
        dilate_half(0, bpost0, cst["clo1"][:], bp128[:], postal0, sgrp=1)
        ar0 = rowsm_p.tile([128, W], BF16, tag="ar0", name="ar0")
        nc.vector.tensor_mul(ar0[:], prealive[0][:], postal0[:])
        alive_store(0, ar0)

        compute_block(5)
        finish_block(0)
        compute_block(6)
        finish_block(1)
        compute_block(7)
        finish_block(2)
        finish_block(3)

        bpost1 = rowsm_p.tile([128, W], BF16, tag="bpost1", name="bpost1")
        post_binary(randrow[1][:], x3row[1][:], d3row[1][:], bpost1)
        postal1 = rowsm_p.tile([128, W], BF16, tag="postal1", name="postal1")
        dilate_half(1, bpost1, cst["bandChi"][:], bpost0[:], postal1, sgrp=1)
        ar1 = rowsm_p.tile([128, W], BF16, tag="ar1", name="ar1")
        nc.vector.tensor_mul(ar1[:], prealive[1][:], postal1[:])
        alive_store(1, ar1)

        for blk in range(4, 8):
            finish_block(blk)

    ctx.close()


# ---------------------------------------------------------------------------

_NC_CACHE = {}


def _get_nc(n_img, reps=1):
    key = (n_img, reps)
    if key not in _NC_CACHE:
        _NC_CACHE[key] = build_program(n_img, reps)
    return _NC_CACHE[key]


def kernel(x, w1, b1, w2, b2, rand_mask):
    x = np.ascontiguousarray(np.asarray(x, np.float32))
    rand_mask = np.ascontiguousarray(np.asarray(rand_mask, np.float32))
    B = x.shape[0]
    n_img = B // N_CORES
    consts = _build_consts(w1, b1, w2, b2)
    cast = {k: np.ascontiguousarray(v.astype(mybir.dt.np(CONST_SPECS[k][1])))
            for k, v in consts.items()}

    nc = _get_nc(n_img)
    in_maps = []
    for k in range(N_CORES):
        sl = slice(k * n_img, (k + 1) * n_img)
        in_maps.append(dict(x=x[sl], rand=rand_mask[sl, 0], **cast))
    res = run_bass_kernel_spmd(nc, in_maps, core_ids=list(range(N_CORES)))
    out = np.concatenate([res.results[k]["out"] for k in range(N_CORES)],
                         axis=0)
    return out.astype(np.float32)

